# revision 60
# baseline (speedup 1.0000x reference)
"""Block sliding-window attention on 8 TRN2 NeuronCores.

Sharding: sequence-parallel. 8 shards = (batch b in {0,1}) x (quarter s in
0..3); each core owns 2048 consecutive tokens of one batch plus a 256-token
K/V halo from the previous quarter (zeros + -inf gate for the first quarter).
No collectives: each core computes its tokens' full output rows.

Per-core pipeline (everything bf16 on the PE: full rate, ~0.7% rel err):
  P1: K^T/Q^T = W^T @ hiddenT (head-transposed) with RoPE fused in (rotate-
      half via SBUF->SBUF partition-swap DMA + 3 in-place DVE ops per head
      row -- DVE is idle in P1), V = hidden @ Wv (natural layout), all staged
      to DRAM scratch. hiddenT streams in token-group DMAs behind the first
      weight tile so the PE starts ~5us in (after a ones-matmul warmup that
      holds the PE at full p-state); weights are host-pre-tiled so each
      head-column's 16 tiles arrive as one contiguous DMA.
  P2 per 256-token chunk (loads ride the Pool queue, prefetched one chunk
      ahead; stores ride the sync queue): per head, S^T = K Q^T per 128-key
      block packed flat in one 2-bank PSUM tile -- the second current-chunk
      key block only computes its live query half -- one exp on ACT
      (scale=1/sqrt(128), -1e30 bias gates the no-previous case), one 0/1
      triangular mask multiply on DVE, denominator via DVE pre-add + one
      all-ones matmul (broadcasts the key-sum across partitions), O^T/den
      share a 2-bank PSUM tile, fast-approx reciprocal + normalize on DVE.
      den+O matmuls are deferred one head-pair behind the scores so exp
      latency hides under the next pair's scores.
  P3: the chunk's 256 output rows vs SBUF-resident Wo, emitted one chunk
      late so the 16-head accumulation never waits on the normalize chain;
      outputs store as bf16 and the host widens to f32.
"""
import sys

try:
    import concourse  # noqa: F401
except ImportError:
    sys.path.insert(0, '/opt/trn_rl_repo')

import ml_dtypes
import numpy as np

import concourse.bacc as bacc
import concourse.mybir as mybir
import concourse.tile as tile
from concourse.bass_utils import run_bass_kernel_spmd

f32 = mybir.dt.float32
AF = mybir.ActivationFunctionType
bf16 = mybir.dt.bfloat16

DIMS = 2048
HEADS = 16
HD = 128           # head dim
WIN = 256          # window / chunk
B, S = 2, 8192
NSH = 4            # seq shards per batch
THETA = 10000.0
ISQ = float(1.0 / np.sqrt(HD))
IB = DIMS // 128   # 16 input-dim blocks


def tok_tiles(n):
    out, a = [], 0
    while a < n:
        w = min(512, n - a)
        out.append((a, w))
        a += w
    return out


def build(nc, T):
    """Emit the per-core program. T = local tokens (multiple of 512)."""
    TH = T + WIN                      # with halo
    NC_ = T // WIN                    # chunks
    HT = nc.dram_tensor("HT", [DIMS, TH], bf16, kind="ExternalInput")
    # WQ/WK pre-tiled on host to [ob, p, ib, o] so each head-column's
    # 16 weight tiles arrive as one fully-contiguous DMA
    WQ = nc.dram_tensor("WQ", [HEADS, 128, IB, 128], bf16,
                        kind="ExternalInput")
    WK = nc.dram_tensor("WK", [HEADS, 128, IB, 128], bf16,
                        kind="ExternalInput")
    WV = nc.dram_tensor("WV", [DIMS, DIMS], bf16, kind="ExternalInput")
    WO = nc.dram_tensor("WO", [DIMS, DIMS], bf16, kind="ExternalInput")
    COS = nc.dram_tensor("COS", [HD, TH], bf16, kind="ExternalInput")
    SINS = nc.dram_tensor("SINS", [HD, TH], bf16, kind="ExternalInput")
    TRI23 = nc.dram_tensor("TRI23", [128, WIN + 128], bf16,
                           kind="ExternalInput")
    PGATE = nc.dram_tensor("PGATE", [128, 1], f32, kind="ExternalInput")
    ONESM = nc.dram_tensor("ONESM", [128, 128], bf16, kind="ExternalInput")
    OUT = nc.dram_tensor("OUT", [T, DIMS], bf16, kind="ExternalOutput")

    QTS = nc.dram_tensor("QTS", [HEADS, HD, T], bf16)    # raw (pre-RoPE) Q^T
    KTS = nc.dram_tensor("KTS", [HEADS, HD, TH], bf16)   # raw K^T (with halo)
    VS = nc.dram_tensor("VS", [TH, DIMS], bf16)          # V natural

    with tile.TileContext(nc) as tc:
        with tc.tile_pool(name="cst", bufs=1) as cst, \
             tc.tile_pool(name="qk", bufs=2) as qk:
            tri23 = cst.tile([128, WIN + 128], bf16)
            pgate = cst.tile([128, 1], f32)
            onesm = cst.tile([128, 128], bf16)
            cosb = cst.tile([128, 1, TH], bf16)
            sinb = cst.tile([128, 1, TH], bf16)
            nc.gpsimd.dma_start(onesm[:], ONESM[:])
            nc.gpsimd.dma_start(tri23[:], TRI23[:])
            nc.gpsimd.dma_start(pgate[:], PGATE[:])
            nc.gpsimd.dma_start(cosb[:, 0], COS[:])
            nc.gpsimd.dma_start(sinb[:, 0], SINS[:])

            # PE warmup while the first hidden-state groups are in flight:
            # keeps the PE continuously busy so the real matmuls start at
            # full clock instead of ramping from the low p-state
            with tc.tile_pool(name="wu", bufs=1, space="PSUM") as wu:
                wps = wu.tile([128, 128], f32)
                NWU = 48
                for i in range(NWU):
                    nc.tensor.matmul(wps[:], onesm[:], onesm[:],
                                     start=(i == 0), stop=(i == NWU - 1))

            # ---------------- P1: projections ----------------
            with tc.tile_pool(name="p1", bufs=1) as p1, \
                 tc.tile_pool(name="wp", bufs=2) as wp, \
                 tc.tile_pool(name="st", bufs=2) as st, \
                 tc.tile_pool(name="pp", bufs=6, space="PSUM") as pp:
                def load_wt(W_, ob):
                    wt = wp.tile([128, IB, 128], bf16, name="w")
                    nc.sync.dma_start(wt[:], W_[ob])
                    return wt

                # first token groups ahead of the first weight tile: the
                # weight tile is the smallest transfer in the chain feeding
                # the first matmul, so it goes last
                ht = p1.tile([128, IB, TH], bf16)
                groups = [(0, 256), (256, 256)] + tok_tiles(TH)[1:]
                for a, w in groups[:2]:
                    nc.sync.dma_start(
                        ht[:, :, a:a + w],
                        HT[:, a:a + w].rearrange("(ib p) t -> p ib t", p=128))
                wt0 = load_wt(WK, 0)
                for a, w in groups[2:]:
                    nc.sync.dma_start(
                        ht[:, :, a:a + w],
                        HT[:, a:a + w].rearrange("(ib p) t -> p ib t", p=128))

                # K^T / Q^T: lhsT = W column block [128in, 128out].
                # RoPE is applied here, once per head row: rotate-half via
                # SBUF->SBUF partition-swap DMA, then 3 in-place DVE ops
                # (DVE is otherwise idle in P1). pos0 = column into COS/SINS.
                def rope_store(stg, DST, ob, t0, tlen):
                    rot = st.tile([128, TH], bf16, name="rot", bufs=1)
                    nc.sync.dma_start(rot[0:64, 0:tlen], stg[64:128, 0:tlen])
                    nc.sync.dma_start(rot[64:128, 0:tlen], stg[0:64, 0:tlen])
                    cs = cosb[:, 0, t0:t0 + tlen]
                    sn = sinb[:, 0, t0:t0 + tlen]
                    nc.vector.tensor_mul(rot[:, 0:tlen], rot[:, 0:tlen], sn)
                    nc.vector.tensor_mul(stg[:, 0:tlen], stg[:, 0:tlen], cs)
                    nc.vector.tensor_add(stg[:, 0:tlen], stg[:, 0:tlen],
                                         rot[:, 0:tlen])
                    nc.sync.dma_start(DST[ob][:, 0:tlen], stg[:, 0:tlen])

                seq = [(WK, KTS, 0, TH, ob) for ob in range(HEADS)] + \
                      [(WQ, QTS, WIN, T, ob) for ob in range(HEADS)]
                wt_next = wt0
                for idx, (W_, DST, t0, tlen, ob) in enumerate(seq):
                    wt = wt_next
                    if idx + 1 < len(seq):
                        nw, _, _, _, nob = seq[idx + 1]
                        wt_next = load_wt(nw, nob)
                    stg = st.tile([128, TH], bf16, name="stg")
                    for a, w in tok_tiles(tlen):
                        ps = pp.tile([128, 512], f32, name="pp")
                        for ib in range(IB):
                            nc.tensor.matmul(
                                ps[:, :w], wt[:, ib],
                                ht[:, ib, t0 + a:t0 + a + w],
                                start=(ib == 0), stop=(ib == IB - 1))
                        nc.scalar.copy(stg[:, a:a + w], ps[:, :w])
                    rope_store(stg, DST, ob, t0, tlen)

                # V natural: lhsT = hT block [128in, 128tok], rhs = Wv rows
                def load_wv(og):
                    wv = wp.tile([128, IB, 512], bf16, name="wv")
                    nc.sync.dma_start(
                        wv[:], WV[:, og * 512:(og + 1) * 512]
                        .rearrange("(ib p) d -> p ib d", p=128))
                    return wv

                wv_next = load_wv(0)
                for og in range(4):
                    wv = wv_next
                    if og + 1 < 4:
                        wv_next = load_wv(og + 1)
                    for tb in range(TH // 128):
                        ps = pp.tile([128, 512], f32, name="pp")
                        for ib in range(IB):
                            nc.tensor.matmul(
                                ps[:], ht[:, ib, tb * 128:(tb + 1) * 128],
                                wv[:, ib, :],
                                start=(ib == 0), stop=(ib == IB - 1))
                        stgv = st.tile([128, 512], bf16, name="stgv")
                        nc.scalar.copy(stgv[:], ps[:])
                        nc.sync.dma_start(
                            VS[tb * 128:(tb + 1) * 128,
                               og * 512:(og + 1) * 512], stgv[:])

            # ---------------- P2 + P3 fused ----------------
            with tc.tile_pool(name="wop", bufs=1) as wop, \
                 tc.tile_pool(name="pbp", bufs=4) as pbp, \
                 tc.tile_pool(name="pad", bufs=2) as padp, \
                 tc.tile_pool(name="pad2", bufs=4) as padp2, \
                 tc.tile_pool(name="ob", bufs=2) as obp, \
                 tc.tile_pool(name="otp", bufs=18) as otp, \
                 tc.tile_pool(name="st3", bufs=2) as st3, \
                 tc.tile_pool(name="ps_s", bufs=2, space="PSUM") as ps_s, \
                 tc.tile_pool(name="ps_po", bufs=1, space="PSUM") as ps_po, \
                 tc.tile_pool(name="ps_p", bufs=2, space="PSUM") as ps_p:

                def kq_issue(SRC, c0, which):
                    t = qk.tile([128, HEADS, WIN], bf16, name=which,
                                bufs=3 if which == "kt" else 2)
                    nc.gpsimd.dma_start(
                        t[:], SRC[:, :, c0:c0 + WIN]
                        .rearrange("h d w -> d h w"))
                    return t

                def v_issue(w0):
                    v = qk.tile([128, 2, DIMS], bf16, name="v", bufs=3)
                    nc.gpsimd.dma_start(
                        v[:], VS[w0:w0 + WIN].rearrange("(tb p) c -> p tb c",
                                                        p=128))
                    return v

                # prologue: halo window + chunk-0 tiles. K/Q loads first
                # (their scratch is ready mid-P1, so these drain early); the
                # V loads and the big Wo load go behind them in the queue.
                kt_prev = kq_issue(KTS, 0, "kt")
                kt_cur = kq_issue(KTS, WIN, "kt")
                qt = kq_issue(QTS, 0, "qt")
                v_prev = v_issue(0)
                v_cur = v_issue(WIN)
                wo = wop.tile([128, IB, DIMS], bf16)
                for nt in range(4):
                    nc.gpsimd.dma_start(
                        wo[:, :, nt * 512:(nt + 1) * 512],
                        WO[:, nt * 512:(nt + 1) * 512]
                        .rearrange("(ib p) d -> p ib d", p=128))

                def p3_emit(c, ots_c):
                    # P3: project chunk c's 256 output rows against Wo.
                    # Called one chunk late so the accumulation never waits
                    # on the freshly-written normalize chain.
                    for tt in range(2):
                        stg = st3.tile([128, DIMS], bf16, name="st3")
                        for nt in range(4):
                            ps = ps_p.tile([128, 512], f32, name="pp3")
                            for h in range(HEADS):
                                nc.tensor.matmul(
                                    ps[:],
                                    ots_c[h // 2][:, h % 2,
                                                  tt * 128:(tt + 1) * 128],
                                    wo[:, h, nt * 512:(nt + 1) * 512],
                                    start=(h == 0), stop=(h == HEADS - 1))
                            nc.vector.tensor_copy(
                                stg[:, nt * 512:(nt + 1) * 512], ps[:])
                        r0 = c * WIN + tt * 128
                        if c == NC_ - 1 and tt == 1:
                            # split the final store so the tail drains sooner
                            nc.sync.dma_start(OUT[r0:r0 + 128, 0:1024],
                                              stg[:, 0:1024])
                            nc.sync.dma_start(OUT[r0:r0 + 128, 1024:2048],
                                              stg[:, 1024:2048])
                        else:
                            nc.sync.dma_start(OUT[r0:r0 + 128, :], stg[:])

                W2 = 2 * WIN
                p3_prev = None
                for c in range(NC_):
                    if c + 1 < NC_:
                        kt_next = kq_issue(KTS, WIN + (c + 1) * WIN, "kt")
                        qt_next = kq_issue(QTS, (c + 1) * WIN, "qt")
                        v_next = v_issue(WIN + (c + 1) * WIN)

                    kts = [kt_prev, kt_prev, kt_cur, kt_cur]
                    vs = [v_prev, v_prev, v_cur, v_cur]
                    ots_c = []
                    pend = None  # (h0, [(h, pb, pa2) x2]) awaiting den+O

                    def den_o(pair):
                        pod = ps_po.tile([128, 4, WIN], f32, name="pod")
                        for i, (h, pb, pa2) in enumerate(pair):
                            nc.tensor.matmul(pod[:, 2 + i], onesm[:], pa2[:],
                                             start=True, stop=True)
                            for kb in range(3):
                                nc.tensor.matmul(
                                    pod[:, i],
                                    vs[kb][:, kb % 2, h * 128:(h + 1) * 128],
                                    pb[:, kb * WIN:(kb + 1) * WIN],
                                    start=(kb == 0), stop=(kb == 2))
                            nc.tensor.matmul(
                                pod[:, i, 128:WIN],
                                vs[3][:, 1, h * 128:(h + 1) * 128],
                                pb[:, 3 * WIN:3 * WIN + 128], start=False,
                                stop=True, skip_group_check=True)
                        rb = obp.tile([128, 2, WIN], f32, name="rb")
                        nc.vector.reciprocal_approx_fast(rb[:], pod[:, 2:4])
                        ot = otp.tile([128, 2, WIN], bf16, name="ot")
                        nc.vector.tensor_mul(ot[:], pod[:, 0:2], rb[:])
                        ots_c.append(ot)

                    for h0 in range(0, HEADS, 2):
                        pair = []
                        for h in (h0, h0 + 1):
                            # scores packed flat in one 2-bank tile; key
                            # block 3 only sees queries 128:256 (queries
                            # 0:128 are fully causal-masked against keys
                            # 128:256), packed at cols 768:896 so a single
                            # exp and a single mask op cover everything
                            ps = ps_s.tile([128, 4 * WIN], f32, name="ps")
                            for kb in range(3):
                                nc.tensor.matmul(
                                    ps[:, kb * WIN:(kb + 1) * WIN],
                                    kts[kb][:, h,
                                            (kb % 2) * 128:(kb % 2) * 128 + 128],
                                    qt[:, h], start=True, stop=True)
                            nc.tensor.matmul(
                                ps[:, 3 * WIN:3 * WIN + 128],
                                kts[3][:, h, 128:256],
                                qt[:, h, 128:WIN], start=True, stop=True)
                            pb = pbp.tile([128, 4 * WIN], bf16, name="pb")
                            if c == 0:
                                nc.scalar.activation(
                                    pb[:, 0:2 * WIN], ps[:, 0:2 * WIN],
                                    AF.Exp, bias=pgate[:], scale=ISQ)
                                nc.scalar.activation(
                                    pb[:, 2 * WIN:3 * WIN + 128],
                                    ps[:, 2 * WIN:3 * WIN + 128],
                                    AF.Exp, scale=ISQ)
                            else:
                                nc.scalar.activation(
                                    pb[:, 0:3 * WIN + 128],
                                    ps[:, 0:3 * WIN + 128], AF.Exp, scale=ISQ)
                            nc.vector.tensor_mul(
                                pb[:, 2 * WIN:3 * WIN + 128],
                                pb[:, 2 * WIN:3 * WIN + 128], tri23[:])
                            # denominator pre-add on DVE (kb3 live half only)
                            pa = padp.tile([128, WIN], bf16, name="pa")
                            nc.vector.tensor_add(pa[:], pb[:, 0:WIN],
                                                 pb[:, WIN:2 * WIN])
                            pa2 = padp2.tile([128, WIN], bf16, name="pa2")
                            nc.vector.tensor_add(pa2[:], pa[:],
                                                 pb[:, 2 * WIN:3 * WIN])
                            nc.vector.tensor_add(
                                pa2[:, 128:WIN], pa2[:, 128:WIN],
                                pb[:, 3 * WIN:3 * WIN + 128])
                            pair.append((h, pb, pa2))
                        if pend is not None:
                            den_o(pend)
                        pend = pair
                    den_o(pend)

                    if p3_prev is not None:
                        p3_emit(c - 1, p3_prev)
                    p3_prev = ots_c
                    if c + 1 < NC_:
                        kt_prev, v_prev = kt_cur, v_cur
                        kt_cur, v_cur, qt = kt_next, v_next, qt_next
                p3_emit(NC_ - 1, p3_prev)
    return nc


def _host_inputs(hidden_states, Wq, Wk, Wv, Wo, T):
    """Build the 8 per-core input maps."""
    TH = T + WIN
    inv_freq = 1.0 / (THETA ** (np.arange(0, HD, 2, dtype=np.float32) / HD))

    qq = np.arange(WIN)[None, :]
    kk = np.arange(128)[:, None]
    # [mask for current-chunk keys 0:128 vs all 256 queries |
    #  mask for keys 128:256 vs the live queries 128:256]
    tri23 = np.concatenate(
        [(qq >= kk), (qq[:, 128:] >= kk + 128)], 1).astype(ml_dtypes.bfloat16)
    onesm_bf = np.ones((128, 128), ml_dtypes.bfloat16)

    Wq, Wk, Wv, Wo = (np.asarray(w, np.float32).astype(ml_dtypes.bfloat16)
                      for w in (Wq, Wk, Wv, Wo))
    # [in, out] -> [ob, p, ib, o] tiles (p = row within 128-input block)
    Wq, Wk = (np.ascontiguousarray(
        w.reshape(IB, 128, HEADS, 128).transpose(2, 1, 0, 3))
        for w in (Wq, Wk))
    in_maps = []
    for core in range(8):
        b, sh = divmod(core, NSH)
        t0 = sh * T
        hs = np.zeros((TH, DIMS), np.float32)
        lo = max(0, t0 - WIN)
        hs[WIN - (t0 - lo):] = hidden_states[b, lo:t0 + T]
        hT = np.ascontiguousarray(hs.T).astype(ml_dtypes.bfloat16)

        pos = np.arange(t0 - WIN, t0 + T, dtype=np.float32)
        f = np.outer(inv_freq, pos)                      # [64, TH]
        cos = np.concatenate([np.cos(f), np.cos(f)], 0)  # [128, TH]
        sin = np.sin(f)
        sins = np.concatenate([-sin, sin], 0)
        pg = np.full((128, 1), -1e30 if sh == 0 else 0.0, np.float32)
        in_maps.append({
            "HT": hT, "WQ": Wq, "WK": Wk, "WV": Wv, "WO": Wo,
            "COS": cos.astype(ml_dtypes.bfloat16),
            "SINS": sins.astype(ml_dtypes.bfloat16),
            "TRI23": tri23, "PGATE": pg, "ONESM": onesm_bf,
        })
    return in_maps


_CACHE = {}


def run(hidden_states, Wq, Wk, Wv, Wo, T=S // NSH, **spmd_kwargs):
    key = T
    if key not in _CACHE:
        nc = bacc.Bacc(None)
        build(nc, T)
        nc.finalize()
        _CACHE[key] = nc
    nc = _CACHE[key]
    in_maps = _host_inputs(hidden_states, Wq, Wk, Wv, Wo, T)
    res = run_bass_kernel_spmd(nc, in_maps, core_ids=list(range(8)), **spmd_kwargs)
    outs = [res.results[i]["OUT"] for i in range(8)]
    full = np.empty((B, NSH * T, DIMS), np.float32)
    for core in range(8):
        b, sh = divmod(core, NSH)
        full[b, sh * T:(sh + 1) * T] = np.asarray(outs[core]).astype(np.float32)
    return full, res


def kernel(hidden_states, Wq, Wk, Wv, Wo):
    out, _ = run(np.asarray(hidden_states), Wq, Wk, Wv, Wo)
    return out


# revision 61
# speedup vs baseline: 1.0030x; 1.0030x over previous
"""Block sliding-window attention on 8 TRN2 NeuronCores.

Sharding: sequence-parallel. 8 shards = (batch b in {0,1}) x (quarter s in
0..3); each core owns 2048 consecutive tokens of one batch plus a 256-token
K/V halo from the previous quarter (zeros + -inf gate for the first quarter).
No collectives: each core computes its tokens' full output rows.

Per-core pipeline (everything bf16 on the PE: full rate, ~0.7% rel err):
  P1: K^T/Q^T = W^T @ hiddenT (head-transposed) with RoPE fused in (rotate-
      half via SBUF->SBUF partition-swap DMA + 3 in-place DVE ops per head
      row -- DVE is idle in P1), V = hidden @ Wv (natural layout), all staged
      to DRAM scratch. hiddenT streams in token-group DMAs behind the first
      weight tile so the PE starts ~5us in (after a ones-matmul warmup that
      holds the PE at full p-state); weights are host-pre-tiled so each
      head-column's 16 tiles arrive as one contiguous DMA.
  P2 per 256-token chunk (loads ride the Pool queue, prefetched one chunk
      ahead; stores ride the sync queue): per head, S^T = K Q^T per 128-key
      block packed flat in one 2-bank PSUM tile -- the second current-chunk
      key block only computes its live query half -- one exp on ACT
      (scale=1/sqrt(128), -1e30 bias gates the no-previous case), one 0/1
      triangular mask multiply on DVE, denominator via DVE pre-add + one
      all-ones matmul (broadcasts the key-sum across partitions), O^T/den
      share a 2-bank PSUM tile, fast-approx reciprocal + normalize on DVE.
      den+O matmuls are deferred one head-pair behind the scores so exp
      latency hides under the next pair's scores.
  P3: the chunk's 256 output rows vs SBUF-resident Wo, emitted one chunk
      late so the 16-head accumulation never waits on the normalize chain;
      outputs store as bf16 and the host widens to f32.
"""
import sys

try:
    import concourse  # noqa: F401
except ImportError:
    sys.path.insert(0, '/opt/trn_rl_repo')

import ml_dtypes
import numpy as np

import concourse.bacc as bacc
import concourse.mybir as mybir
import concourse.tile as tile
from concourse.bass_utils import run_bass_kernel_spmd

f32 = mybir.dt.float32
AF = mybir.ActivationFunctionType
bf16 = mybir.dt.bfloat16

DIMS = 2048
HEADS = 16
HD = 128           # head dim
WIN = 256          # window / chunk
B, S = 2, 8192
NSH = 4            # seq shards per batch
THETA = 10000.0
ISQ = float(1.0 / np.sqrt(HD))
IB = DIMS // 128   # 16 input-dim blocks


def tok_tiles(n):
    out, a = [], 0
    while a < n:
        w = min(512, n - a)
        out.append((a, w))
        a += w
    return out


def build(nc, T):
    """Emit the per-core program. T = local tokens (multiple of 512)."""
    TH = T + WIN                      # with halo
    NC_ = T // WIN                    # chunks
    HT = nc.dram_tensor("HT", [DIMS, TH], bf16, kind="ExternalInput")
    # WQ/WK pre-tiled on host to [ob, p, ib, o] so each head-column's
    # 16 weight tiles arrive as one fully-contiguous DMA
    WQ = nc.dram_tensor("WQ", [HEADS, 128, IB, 128], bf16,
                        kind="ExternalInput")
    WK = nc.dram_tensor("WK", [HEADS, 128, IB, 128], bf16,
                        kind="ExternalInput")
    WV = nc.dram_tensor("WV", [DIMS, DIMS], bf16, kind="ExternalInput")
    WO = nc.dram_tensor("WO", [DIMS, DIMS], bf16, kind="ExternalInput")
    COS = nc.dram_tensor("COS", [HD, TH], bf16, kind="ExternalInput")
    SINS = nc.dram_tensor("SINS", [HD, TH], bf16, kind="ExternalInput")
    TRI23 = nc.dram_tensor("TRI23", [128, WIN + 128], bf16,
                           kind="ExternalInput")
    PGATE = nc.dram_tensor("PGATE", [128, 1], f32, kind="ExternalInput")
    ONESM = nc.dram_tensor("ONESM", [128, 128], bf16, kind="ExternalInput")
    OUT = nc.dram_tensor("OUT", [T, DIMS], bf16, kind="ExternalOutput")

    QTS = nc.dram_tensor("QTS", [HEADS, HD, T], bf16)    # raw (pre-RoPE) Q^T
    KTS = nc.dram_tensor("KTS", [HEADS, HD, TH], bf16)   # raw K^T (with halo)
    VS = nc.dram_tensor("VS", [TH, DIMS], bf16)          # V natural

    with tile.TileContext(nc) as tc:
        with tc.tile_pool(name="cst", bufs=1) as cst, \
             tc.tile_pool(name="qk", bufs=2) as qk:
            tri23 = cst.tile([128, WIN + 128], bf16)
            pgate = cst.tile([128, 1], f32)
            onesm = cst.tile([128, 128], bf16)
            cosb = cst.tile([128, 1, TH], bf16)
            sinb = cst.tile([128, 1, TH], bf16)
            nc.gpsimd.dma_start(onesm[:], ONESM[:])
            nc.gpsimd.dma_start(tri23[:], TRI23[:])
            nc.gpsimd.dma_start(pgate[:], PGATE[:])
            nc.gpsimd.dma_start(cosb[:, 0], COS[:])
            nc.gpsimd.dma_start(sinb[:, 0], SINS[:])

            # PE warmup while the first hidden-state groups are in flight:
            # keeps the PE continuously busy so the real matmuls start at
            # full clock instead of ramping from the low p-state
            with tc.tile_pool(name="wu", bufs=1, space="PSUM") as wu:
                wps = wu.tile([128, 128], f32)
                NWU = 48
                for i in range(NWU):
                    nc.tensor.matmul(wps[:], onesm[:], onesm[:],
                                     start=(i == 0), stop=(i == NWU - 1))

            # ---------------- P1: projections ----------------
            with tc.tile_pool(name="p1", bufs=1) as p1, \
                 tc.tile_pool(name="wp", bufs=2) as wp, \
                 tc.tile_pool(name="st", bufs=2) as st, \
                 tc.tile_pool(name="pp", bufs=6, space="PSUM") as pp:
                def load_wt(W_, ob):
                    wt = wp.tile([128, IB, 128], bf16, name="w")
                    nc.sync.dma_start(wt[:], W_[ob])
                    return wt

                # first token groups ahead of the first weight tile: the
                # weight tile is the smallest transfer in the chain feeding
                # the first matmul, so it goes last
                ht = p1.tile([128, IB, TH], bf16)
                groups = [(0, 256), (256, 256)] + tok_tiles(TH)[1:]
                for a, w in groups[:2]:
                    nc.sync.dma_start(
                        ht[:, :, a:a + w],
                        HT[:, a:a + w].rearrange("(ib p) t -> p ib t", p=128))
                wt0 = load_wt(WK, 0)
                for a, w in groups[2:]:
                    nc.sync.dma_start(
                        ht[:, :, a:a + w],
                        HT[:, a:a + w].rearrange("(ib p) t -> p ib t", p=128))

                # K^T / Q^T: lhsT = W column block [128in, 128out].
                # RoPE is applied here, once per head row: rotate-half via
                # SBUF->SBUF partition-swap DMA, then 3 in-place DVE ops
                # (DVE is otherwise idle in P1). pos0 = column into COS/SINS.
                def rope_store(stg, DST, ob, t0, tlen):
                    rot = st.tile([128, TH], bf16, name="rot", bufs=1)
                    nc.sync.dma_start(rot[0:64, 0:tlen], stg[64:128, 0:tlen])
                    nc.sync.dma_start(rot[64:128, 0:tlen], stg[0:64, 0:tlen])
                    cs = cosb[:, 0, t0:t0 + tlen]
                    sn = sinb[:, 0, t0:t0 + tlen]
                    nc.vector.tensor_mul(rot[:, 0:tlen], rot[:, 0:tlen], sn)
                    nc.vector.tensor_mul(stg[:, 0:tlen], stg[:, 0:tlen], cs)
                    nc.vector.tensor_add(stg[:, 0:tlen], stg[:, 0:tlen],
                                         rot[:, 0:tlen])
                    nc.sync.dma_start(DST[ob][:, 0:tlen], stg[:, 0:tlen])

                seq = [(WK, KTS, 0, TH, ob) for ob in range(HEADS)] + \
                      [(WQ, QTS, WIN, T, ob) for ob in range(HEADS)]
                wt_next = wt0
                for idx, (W_, DST, t0, tlen, ob) in enumerate(seq):
                    wt = wt_next
                    if idx + 1 < len(seq):
                        nw, _, _, _, nob = seq[idx + 1]
                        wt_next = load_wt(nw, nob)
                    stg = st.tile([128, TH], bf16, name="stg")
                    for a, w in tok_tiles(tlen):
                        ps = pp.tile([128, 512], f32, name="pp")
                        for ib in range(IB):
                            nc.tensor.matmul(
                                ps[:, :w], wt[:, ib],
                                ht[:, ib, t0 + a:t0 + a + w],
                                start=(ib == 0), stop=(ib == IB - 1))
                        nc.scalar.copy(stg[:, a:a + w], ps[:, :w])
                    rope_store(stg, DST, ob, t0, tlen)

                # V natural: lhsT = hT block [128in, 128tok], rhs = Wv rows
                def load_wv(og):
                    wv = wp.tile([128, IB, 512], bf16, name="wv")
                    nc.sync.dma_start(
                        wv[:], WV[:, og * 512:(og + 1) * 512]
                        .rearrange("(ib p) d -> p ib d", p=128))
                    return wv

                wv_next = load_wv(0)
                for og in range(4):
                    wv = wv_next
                    if og + 1 < 4:
                        wv_next = load_wv(og + 1)
                    for tb in range(TH // 128):
                        ps = pp.tile([128, 512], f32, name="pp")
                        for ib in range(IB):
                            nc.tensor.matmul(
                                ps[:], ht[:, ib, tb * 128:(tb + 1) * 128],
                                wv[:, ib, :],
                                start=(ib == 0), stop=(ib == IB - 1))
                        stgv = st.tile([128, 512], bf16, name="stgv")
                        nc.scalar.copy(stgv[:], ps[:])
                        nc.sync.dma_start(
                            VS[tb * 128:(tb + 1) * 128,
                               og * 512:(og + 1) * 512], stgv[:])

            # ---------------- P2 + P3 fused ----------------
            with tc.tile_pool(name="wop", bufs=1) as wop, \
                 tc.tile_pool(name="pbp", bufs=5) as pbp, \
                 tc.tile_pool(name="pad", bufs=2) as padp, \
                 tc.tile_pool(name="pad2", bufs=5) as padp2, \
                 tc.tile_pool(name="ob", bufs=3) as obp, \
                 tc.tile_pool(name="otp", bufs=18) as otp, \
                 tc.tile_pool(name="st3", bufs=2) as st3, \
                 tc.tile_pool(name="ps_s", bufs=2, space="PSUM") as ps_s, \
                 tc.tile_pool(name="ps_po", bufs=1, space="PSUM") as ps_po, \
                 tc.tile_pool(name="ps_p", bufs=2, space="PSUM") as ps_p:

                def kq_issue(SRC, c0, which):
                    t = qk.tile([128, HEADS, WIN], bf16, name=which,
                                bufs=3 if which == "kt" else 2)
                    nc.gpsimd.dma_start(
                        t[:], SRC[:, :, c0:c0 + WIN]
                        .rearrange("h d w -> d h w"))
                    return t

                def v_issue(w0):
                    v = qk.tile([128, 2, DIMS], bf16, name="v", bufs=3)
                    nc.gpsimd.dma_start(
                        v[:], VS[w0:w0 + WIN].rearrange("(tb p) c -> p tb c",
                                                        p=128))
                    return v

                # prologue: halo window + chunk-0 tiles. K/Q loads first
                # (their scratch is ready mid-P1, so these drain early); the
                # V loads and the big Wo load go behind them in the queue.
                kt_prev = kq_issue(KTS, 0, "kt")
                kt_cur = kq_issue(KTS, WIN, "kt")
                qt = kq_issue(QTS, 0, "qt")
                v_prev = v_issue(0)
                v_cur = v_issue(WIN)
                wo = wop.tile([128, IB, DIMS], bf16)
                for nt in range(4):
                    nc.gpsimd.dma_start(
                        wo[:, :, nt * 512:(nt + 1) * 512],
                        WO[:, nt * 512:(nt + 1) * 512]
                        .rearrange("(ib p) d -> p ib d", p=128))

                def p3_emit(c, ots_c):
                    # P3: project chunk c's 256 output rows against Wo.
                    # Called one chunk late so the accumulation never waits
                    # on the freshly-written normalize chain.
                    for tt in range(2):
                        stg = st3.tile([128, DIMS], bf16, name="st3")
                        for nt in range(4):
                            ps = ps_p.tile([128, 512], f32, name="pp3")
                            for h in range(HEADS):
                                nc.tensor.matmul(
                                    ps[:],
                                    ots_c[h // 2][:, h % 2,
                                                  tt * 128:(tt + 1) * 128],
                                    wo[:, h, nt * 512:(nt + 1) * 512],
                                    start=(h == 0), stop=(h == HEADS - 1))
                            nc.vector.tensor_copy(
                                stg[:, nt * 512:(nt + 1) * 512], ps[:])
                        r0 = c * WIN + tt * 128
                        if c == NC_ - 1 and tt == 1:
                            # split the final store so the tail drains sooner
                            nc.sync.dma_start(OUT[r0:r0 + 128, 0:1024],
                                              stg[:, 0:1024])
                            nc.sync.dma_start(OUT[r0:r0 + 128, 1024:2048],
                                              stg[:, 1024:2048])
                        else:
                            nc.sync.dma_start(OUT[r0:r0 + 128, :], stg[:])

                W2 = 2 * WIN
                p3_prev = None
                for c in range(NC_):
                    if c + 1 < NC_:
                        kt_next = kq_issue(KTS, WIN + (c + 1) * WIN, "kt")
                        qt_next = kq_issue(QTS, (c + 1) * WIN, "qt")
                        v_next = v_issue(WIN + (c + 1) * WIN)

                    kts = [kt_prev, kt_prev, kt_cur, kt_cur]
                    vs = [v_prev, v_prev, v_cur, v_cur]
                    ots_c = []
                    pend = None  # (h0, [(h, pb, pa2) x2]) awaiting den+O

                    def den_o(pair):
                        pod = ps_po.tile([128, 4, WIN], f32, name="pod")
                        for i, (h, pb, pa2) in enumerate(pair):
                            nc.tensor.matmul(pod[:, 2 + i], onesm[:], pa2[:],
                                             start=True, stop=True)
                            for kb in range(3):
                                nc.tensor.matmul(
                                    pod[:, i],
                                    vs[kb][:, kb % 2, h * 128:(h + 1) * 128],
                                    pb[:, kb * WIN:(kb + 1) * WIN],
                                    start=(kb == 0), stop=(kb == 2))
                            nc.tensor.matmul(
                                pod[:, i, 128:WIN],
                                vs[3][:, 1, h * 128:(h + 1) * 128],
                                pb[:, 3 * WIN:3 * WIN + 128], start=False,
                                stop=True, skip_group_check=True)
                        rb = obp.tile([128, 2, WIN], f32, name="rb")
                        nc.vector.reciprocal_approx_fast(rb[:], pod[:, 2:4])
                        ot = otp.tile([128, 2, WIN], bf16, name="ot")
                        nc.vector.tensor_mul(ot[:], pod[:, 0:2], rb[:])
                        ots_c.append(ot)

                    for h0 in range(0, HEADS, 2):
                        pair = []
                        for h in (h0, h0 + 1):
                            # scores packed flat in one 2-bank tile; key
                            # block 3 only sees queries 128:256 (queries
                            # 0:128 are fully causal-masked against keys
                            # 128:256), packed at cols 768:896 so a single
                            # exp and a single mask op cover everything
                            ps = ps_s.tile([128, 4 * WIN], f32, name="ps")
                            for kb in range(3):
                                nc.tensor.matmul(
                                    ps[:, kb * WIN:(kb + 1) * WIN],
                                    kts[kb][:, h,
                                            (kb % 2) * 128:(kb % 2) * 128 + 128],
                                    qt[:, h], start=True, stop=True)
                            nc.tensor.matmul(
                                ps[:, 3 * WIN:3 * WIN + 128],
                                kts[3][:, h, 128:256],
                                qt[:, h, 128:WIN], start=True, stop=True)
                            pb = pbp.tile([128, 4 * WIN], bf16, name="pb")
                            if c == 0:
                                nc.scalar.activation(
                                    pb[:, 0:2 * WIN], ps[:, 0:2 * WIN],
                                    AF.Exp, bias=pgate[:], scale=ISQ)
                                nc.scalar.activation(
                                    pb[:, 2 * WIN:3 * WIN + 128],
                                    ps[:, 2 * WIN:3 * WIN + 128],
                                    AF.Exp, scale=ISQ)
                            else:
                                nc.scalar.activation(
                                    pb[:, 0:3 * WIN + 128],
                                    ps[:, 0:3 * WIN + 128], AF.Exp, scale=ISQ)
                            nc.vector.tensor_mul(
                                pb[:, 2 * WIN:3 * WIN + 128],
                                pb[:, 2 * WIN:3 * WIN + 128], tri23[:])
                            # denominator pre-add on DVE (kb3 live half only)
                            pa = padp.tile([128, WIN], bf16, name="pa")
                            nc.vector.tensor_add(pa[:], pb[:, 0:WIN],
                                                 pb[:, WIN:2 * WIN])
                            pa2 = padp2.tile([128, WIN], bf16, name="pa2")
                            nc.vector.tensor_add(pa2[:], pa[:],
                                                 pb[:, 2 * WIN:3 * WIN])
                            nc.vector.tensor_add(
                                pa2[:, 128:WIN], pa2[:, 128:WIN],
                                pb[:, 3 * WIN:3 * WIN + 128])
                            pair.append((h, pb, pa2))
                        if pend is not None:
                            den_o(pend)
                        pend = pair
                    den_o(pend)

                    if p3_prev is not None:
                        p3_emit(c - 1, p3_prev)
                    p3_prev = ots_c
                    if c + 1 < NC_:
                        kt_prev, v_prev = kt_cur, v_cur
                        kt_cur, v_cur, qt = kt_next, v_next, qt_next
                p3_emit(NC_ - 1, p3_prev)
    return nc


def _host_inputs(hidden_states, Wq, Wk, Wv, Wo, T):
    """Build the 8 per-core input maps."""
    TH = T + WIN
    inv_freq = 1.0 / (THETA ** (np.arange(0, HD, 2, dtype=np.float32) / HD))

    qq = np.arange(WIN)[None, :]
    kk = np.arange(128)[:, None]
    # [mask for current-chunk keys 0:128 vs all 256 queries |
    #  mask for keys 128:256 vs the live queries 128:256]
    tri23 = np.concatenate(
        [(qq >= kk), (qq[:, 128:] >= kk + 128)], 1).astype(ml_dtypes.bfloat16)
    onesm_bf = np.ones((128, 128), ml_dtypes.bfloat16)

    Wq, Wk, Wv, Wo = (np.asarray(w, np.float32).astype(ml_dtypes.bfloat16)
                      for w in (Wq, Wk, Wv, Wo))
    # [in, out] -> [ob, p, ib, o] tiles (p = row within 128-input block)
    Wq, Wk = (np.ascontiguousarray(
        w.reshape(IB, 128, HEADS, 128).transpose(2, 1, 0, 3))
        for w in (Wq, Wk))
    in_maps = []
    for core in range(8):
        b, sh = divmod(core, NSH)
        t0 = sh * T
        hs = np.zeros((TH, DIMS), np.float32)
        lo = max(0, t0 - WIN)
        hs[WIN - (t0 - lo):] = hidden_states[b, lo:t0 + T]
        hT = np.ascontiguousarray(hs.T).astype(ml_dtypes.bfloat16)

        pos = np.arange(t0 - WIN, t0 + T, dtype=np.float32)
        f = np.outer(inv_freq, pos)                      # [64, TH]
        cos = np.concatenate([np.cos(f), np.cos(f)], 0)  # [128, TH]
        sin = np.sin(f)
        sins = np.concatenate([-sin, sin], 0)
        pg = np.full((128, 1), -1e30 if sh == 0 else 0.0, np.float32)
        in_maps.append({
            "HT": hT, "WQ": Wq, "WK": Wk, "WV": Wv, "WO": Wo,
            "COS": cos.astype(ml_dtypes.bfloat16),
            "SINS": sins.astype(ml_dtypes.bfloat16),
            "TRI23": tri23, "PGATE": pg, "ONESM": onesm_bf,
        })
    return in_maps


_CACHE = {}


def run(hidden_states, Wq, Wk, Wv, Wo, T=S // NSH, **spmd_kwargs):
    key = T
    if key not in _CACHE:
        nc = bacc.Bacc(None)
        build(nc, T)
        nc.finalize()
        _CACHE[key] = nc
    nc = _CACHE[key]
    in_maps = _host_inputs(hidden_states, Wq, Wk, Wv, Wo, T)
    res = run_bass_kernel_spmd(nc, in_maps, core_ids=list(range(8)), **spmd_kwargs)
    outs = [res.results[i]["OUT"] for i in range(8)]
    full = np.empty((B, NSH * T, DIMS), np.float32)
    for core in range(8):
        b, sh = divmod(core, NSH)
        full[b, sh * T:(sh + 1) * T] = np.asarray(outs[core]).astype(np.float32)
    return full, res


def kernel(hidden_states, Wq, Wk, Wv, Wo):
    out, _ = run(np.asarray(hidden_states), Wq, Wk, Wv, Wo)
    return out
